# revision 2
# baseline (speedup 1.0000x reference)
"""Trainium2 Bass kernel for nn_MixerBlock (gnn_message_passing).

Sharding: 8 cores each own a slice of N/8 = 1536 nodes, with ALL 4 batches
and ALL 256 channels local. Every phase (LayerNorms, Chebyshev weighted sums,
convs, token/channel mixing) is core-local; the only cross-core traffic is
an 8-way AllGather of the [N, 4*256] bf16 Chebyshev state table per chain
level (10 total) plus one small AllReduce for the token-mix hidden.

Each 8-neighbor gather descriptor moves one 2048-byte table row (4 batches x
256 ch), so SWDGE descriptor generation is 12288 descs/level/core, 8x fewer
than a batch-sharded layout.
"""
import numpy as np
import ml_dtypes

bf16 = ml_dtypes.bfloat16

B, N, C, K, NB, T, CHID = 4, 12288, 256, 6, 8, 512, 1024
NCORES = 8
NOWN = N // NCORES          # 1536
NT = NOWN // 128            # 12
ROWF = B * C                # 1024
EPS = 1e-6

_cache = {}


def _u16(x):
    return np.ascontiguousarray(x).view(np.uint16)


def _wrap_idx(idxlist):
    num = idxlist.size
    base = idxlist.astype(np.int16).reshape(num // 16, 16).T
    out = np.zeros((128, num // 16), np.int16)
    for g8 in range(8):
        out[16 * g8:16 * g8 + 16, :] = base
    return out


def host_prep(inputs):
    maps = np.asarray(inputs["maps"], np.float32)
    idx = np.asarray(inputs["neigh_idx"], np.int32)
    w = np.asarray(inputs["neigh_w"], np.float32)
    diag = np.asarray(inputs["diag_w"], np.float32)

    conv_w = [np.asarray(inputs["conv1_w"], np.float32),
              np.asarray(inputs["conv2_w"], np.float32)]
    convb = np.stack([
        np.broadcast_to(np.asarray(inputs["conv1_b"], np.float32), (128, C)),
        np.broadcast_to(np.asarray(inputs["conv2_b"], np.float32), (128, C)),
    ])
    tok_w1 = np.asarray(inputs["tok_w1"], np.float32)
    tok_b1 = np.asarray(inputs["tok_b1"], np.float32)
    tok_w2 = np.asarray(inputs["tok_w2"], np.float32)
    tok_b2 = np.asarray(inputs["tok_b2"], np.float32)
    ch_w1 = np.asarray(inputs["ch_w1"], np.float32)
    ch_b1 = np.asarray(inputs["ch_b1"], np.float32)
    ch_w2 = np.asarray(inputs["ch_w2"], np.float32)
    ch_b2 = np.asarray(inputs["ch_b2"], np.float32)

    ln_trivial = {}
    for nm in ("ln1", "ln2", "ln3", "ln4"):
        sc = np.asarray(inputs[nm + "_scale"], np.float32)
        bi = np.asarray(inputs[nm + "_bias"], np.float32)
        ln_trivial[nm] = bool(np.allclose(sc, 1.0) and np.allclose(bi, 0.0))
    lnsc = np.stack([np.broadcast_to(
        np.asarray(inputs[nm + "_scale"], np.float32), (128, C))
        for nm in ("ln1", "ln2", "ln3", "ln4")])
    lnbi = np.stack([np.broadcast_to(
        np.asarray(inputs[nm + "_bias"], np.float32), (128, C))
        for nm in ("ln1", "ln2", "ln3", "ln4")])

    cw = np.stack([np.ascontiguousarray(
        cwx.reshape(K, 2, 128, C).transpose(2, 0, 1, 3).reshape(128, 2 * K, C))
        for cwx in conv_w]).astype(bf16)
    chw1 = np.ascontiguousarray(
        ch_w1.reshape(2, 128, CHID).transpose(1, 0, 2)).astype(bf16)
    chb1 = np.ascontiguousarray(ch_b1.reshape(8, 128).T)
    chw2 = np.ascontiguousarray(
        ch_w2.reshape(8, 128, C).transpose(1, 0, 2)).astype(bf16)
    chb2bc = np.ascontiguousarray(np.broadcast_to(ch_b2, (128, C)))
    tokb1T = np.ascontiguousarray(tok_b1.reshape(4, 128).T)

    tt, gg, hh, ii, kk = np.meshgrid(
        np.arange(NT), np.arange(4), np.arange(2), np.arange(32),
        np.arange(4), indexing="ij")
    nloc = (tt * 128 + gg * 32 + ii).ravel()
    kidx = (hh * 4 + kk).ravel()
    rows = (ii * 4 + kk).ravel()
    cols = (tt * 256 + (gg * 2 + hh) * 32 + ii).ravel()

    in_maps = []
    for q in range(NCORES):
        own = slice(q * NOWN, (q + 1) * NOWN)
        maps_slab = np.ascontiguousarray(
            maps[:, own, :].transpose(1, 0, 2).reshape(NOWN, ROWF))
        idxwrap = _wrap_idx(idx[q * NOWN + nloc, kidx])
        wblk = np.zeros((128, NT * 256), np.float32)
        wblk[rows, cols] = w[q * NOWN + nloc, kidx]
        wblk = wblk.astype(bf16)
        dloc = diag[own].astype(bf16).astype(np.float32)
        m = {
            "maps_slab": maps_slab,
            "idxwrap": idxwrap,
            "wblk": _u16(wblk),
            "d1": np.ascontiguousarray(dloc.reshape(NT, 128).T),
            "d2": np.ascontiguousarray((2.0 * dloc).reshape(NT, 128).T),
            "negI": _u16((-0.25 * np.eye(128)).astype(bf16)),
            "zI": _u16(np.zeros((128, 128), bf16)),
            "cw": _u16(cw),
            "convb": convb,
            "lnsc": np.ascontiguousarray(lnsc),
            "lnbi": np.ascontiguousarray(lnbi),
            "tokw1": _u16(np.ascontiguousarray(
                tok_w1[own].reshape(NT, 128, T).transpose(1, 0, 2)).astype(bf16)),
            "tokb1T": tokb1T,
            "tokw2": _u16(np.ascontiguousarray(
                tok_w2[:, own].reshape(4, 128, NOWN).transpose(1, 0, 2)).astype(bf16)),
            "tokb2c": np.ascontiguousarray(tok_b2[own].reshape(NT, 128).T),
            "chw1": _u16(chw1), "chb1": chb1,
            "chw2": _u16(chw2), "chb2bc": chb2bc,
        }
        in_maps.append(m)
    return in_maps, ln_trivial


def build_nc(num_devices, ln_trivial, native_gelu=True):
    import concourse.bass as bass
    import concourse.bacc as bacc
    import concourse.mybir as mybir
    import concourse.tile as tile

    dt = mybir.dt
    AF = mybir.ActivationFunctionType
    OP = mybir.AluOpType

    nc = bacc.Bacc("TRN2", target_bir_lowering=False, debug=False,
                   num_devices=num_devices)

    def din(name, shape, dtype):
        return nc.dram_tensor(name, shape, dtype, kind="ExternalInput")

    maps_d = din("maps_slab", [NOWN, ROWF], dt.float32)
    idx_d = din("idxwrap", [128, NOWN * NB // 16], dt.int16)
    wblk_d = din("wblk", [128, NT * 256], dt.uint16)
    d1_d = din("d1", [128, NT], dt.float32)
    d2_d = din("d2", [128, NT], dt.float32)
    negI_d = din("negI", [128, 128], dt.uint16)
    zI_d = din("zI", [128, 128], dt.uint16)
    cw_d = din("cw", [2, 128, 2 * K, C], dt.uint16)
    convb_d = din("convb", [2, 128, C], dt.float32)
    lnsc_d = din("lnsc", [4, 128, C], dt.float32)
    lnbi_d = din("lnbi", [4, 128, C], dt.float32)
    tokw1_d = din("tokw1", [128, NT, T], dt.uint16)
    tokb1T_d = din("tokb1T", [128, 4], dt.float32)
    tokw2_d = din("tokw2", [128, 4, NOWN], dt.uint16)
    tokb2c_d = din("tokb2c", [128, NT], dt.float32)
    chw1_d = din("chw1", [128, 2, CHID], dt.uint16)
    chb1_d = din("chb1", [128, 8], dt.float32)
    chw2_d = din("chw2", [128, 8, C], dt.uint16)
    chb2bc_d = din("chb2bc", [128, C], dt.float32)

    outp = nc.dram_tensor("outp", [NOWN, ROWF], dt.float32, kind="ExternalOutput")

    RG = [list(range(num_devices))]

    with tile.TileContext(nc) as tc:
        with tc.tile_pool(name="dram", bufs=1, space="DRAM") as dram, \
             tc.tile_pool(name="const", bufs=1) as const:
            # ------- DRAM scratch -------
            sl = [dram.tile([NOWN, ROWF], dt.bfloat16, tag=f"sl{j}",
                            name=f"sl{j}") for j in range(K)]
            sl8 = [dram.tile([NOWN, ROWF], dt.float8e3, tag=f"sl8_{j}",
                             name=f"sl8_{j}") for j in range(K - 1)]
            tb = [dram.tile([num_devices, NOWN, ROWF], dt.float8e3,
                            addr_space="Shared", tag=f"tb{i}", name=f"tb{i}")
                  for i in range(10)]
            x1d = dram.tile([NOWN, ROWF], dt.float32, tag="x1d", name="x1d")
            z4d = dram.tile([NOWN, ROWF], dt.bfloat16, tag="z4d", name="z4d")
            arin = dram.tile([128, 4096], dt.float32, tag="arin", name="arin")
            arout = dram.tile([128, 4096], dt.float32, addr_space="Shared",
                              tag="arout", name="arout")

            sl_t = [s[:].rearrange("(t p) c -> p t c", p=128) for s in sl]
            sl8_t = [s[:].rearrange("(t p) c -> p t c", p=128) for s in sl8]
            gsem = nc.alloc_semaphore("gsem")
            maps_t = maps_d.ap().rearrange("(t p) c -> p t c", p=128)
            x1_t = x1d[:].rearrange("(t p) c -> p t c", p=128)
            outp_t = outp.ap().rearrange("(t p) c -> p t c", p=128)

            # ------- persistent SBUF constants -------
            idxt = const.tile([128, NOWN * NB // 16], dt.int16)
            nc.sync.dma_start(idxt[:], idx_d.ap())
            wblk = const.tile([128, NT * 256], dt.bfloat16)
            nc.sync.dma_start(wblk[:], wblk_d.ap().bitcast(dt.bfloat16))
            d1 = const.tile([128, NT], dt.float32)
            nc.sync.dma_start(d1[:], d1_d.ap())
            d2 = const.tile([128, NT], dt.float32)
            nc.sync.dma_start(d2[:], d2_d.ap())
            nI = const.tile([128, 128], dt.bfloat16)
            nc.sync.dma_start(nI[:], negI_d.ap().bitcast(dt.bfloat16))
            zI = const.tile([128, 128], dt.bfloat16)
            nc.sync.dma_start(zI[:], zI_d.ap().bitcast(dt.bfloat16))
            eps_col = const.tile([128, 1], dt.float32)
            nc.vector.memset(eps_col[:], EPS)

            def gelu_act(out_ap, in_ap, sb, bias=0.0):
                if native_gelu:
                    nc.scalar.activation(out_ap, in_ap, AF.Gelu_apprx_tanh,
                                         bias=bias)
                else:
                    sgt = sb.tile(list(in_ap.shape), dt.float32, tag="gelu_sg",
                                  name="gelu_sg")
                    nc.scalar.activation(sgt[:], in_ap, AF.Sigmoid,
                                         scale=1.702, bias=bias)
                    tmp = sb.tile(list(in_ap.shape), dt.float32, tag="gelu_t",
                                  name="gelu_t")
                    if bias is not None and not isinstance(bias, float):
                        nc.vector.tensor_scalar(tmp[:], in_ap, bias, None,
                                                OP.add)
                        nc.vector.tensor_tensor(out_ap, tmp[:], sgt[:], OP.mult)
                    else:
                        nc.vector.tensor_tensor(out_ap, in_ap, sgt[:], OP.mult)

            def ln_norm(xt_ap, out_ap, ln_i, triv, sb, sc=None, bi=None):
                """LayerNorm over 256-ch free slice: out = (x-m)/sd [*sc+bi]."""
                st = sb.tile([128, 6], dt.float32, tag="lnst", name="lnst")
                nc.vector.bn_stats(st[:], xt_ap)
                ag = sb.tile([128, 2], dt.float32, tag="lnag", name="lnag")
                nc.vector.bn_aggr(ag[:], st[:])
                sd = sb.tile([128, 1], dt.float32, tag="lnsd", name="lnsd")
                nc.scalar.activation(sd[:], ag[:, 1:2], AF.Sqrt, bias=eps_col[:])
                rs = sb.tile([128, 1], dt.float32, tag="lnrs", name="lnrs")
                nc.vector.reciprocal(rs[:], sd[:])
                zt = sb.tile([128, C], dt.float32, tag="lnzt", name="lnzt")
                nc.vector.tensor_scalar(zt[:], xt_ap, ag[:, 0:1], rs[:],
                                        OP.subtract, OP.mult)
                if not triv:
                    nc.vector.tensor_tensor(zt[:], zt[:], sc[:], OP.mult)
                    nc.vector.tensor_tensor(zt[:], zt[:], bi[:], OP.add)
                nc.vector.tensor_copy(out_ap, zt[:])

            def load_ln_consts(ln_i, triv, pool):
                if triv:
                    return None, None
                sc = pool.tile([128, C], dt.float32, tag=f"lsc{ln_i}",
                               name=f"lsc{ln_i}")
                nc.sync.dma_start(sc[:], lnsc_d.ap()[ln_i])
                bi = pool.tile([128, C], dt.float32, tag=f"lbi{ln_i}",
                               name=f"lbi{ln_i}")
                nc.sync.dma_start(bi[:], lnbi_d.ap()[ln_i])
                return sc, bi

            def run_chain(ring, base, sb, gbp, psp):
                """ring[0] holds t0; sl[0]/sl8[0] already written. Runs
                levels 1..5; AllGathers the fp8 slab, preps all 12 gather
                descriptor sets during the collective, then triggers."""
                for j in range(1, K):
                    nc.gpsimd.collective_compute(
                        "AllGather", mybir.AluOpType.bypass, replica_groups=RG,
                        ins=[sl8[j - 1].opt()], outs=[tb[base + j - 1].opt()])
                    tflat = tb[base + j - 1][:].rearrange("r n c -> (r n) c")
                    dcol = d2 if j >= 2 else d1
                    uscale = 4.0 if j >= 2 else 2.0
                    rin = ring[(j - 1) % 3]
                    rpv = ring[(j - 2) % 3]
                    rout = ring[j % 3]
                    for t in range(NT):
                        gb = gbp.tile([128, 8, ROWF], dt.float8e3, tag="gb",
                                      name="gb")
                        nc.gpsimd.dma_gather(
                            out_ap=gb[:],
                            in_ap=tflat,
                            idxs_ap=idxt[:, t * 64:(t + 1) * 64],
                            num_idxs=1024,
                            num_idxs_reg=1024,
                            elem_size=ROWF,
                            single_packet=True,
                        )
                        for hf in range(2):
                            fs = slice(hf * 512, (hf + 1) * 512)
                            ps = psp.tile([128, 512], dt.float32, tag="cps",
                                          name="cps")
                            if j >= 2:
                                nc.tensor.matmul(ps[:], nI[:], rpv[:, t, fs],
                                                 start=True, stop=True)
                            else:
                                nc.tensor.matmul(ps[:], zI[:], rin[:, t, fs],
                                                 start=True, stop=True)
                            for g in range(4):
                                for h in range(2):
                                    wc = t * 256 + (g * 2 + h) * 32
                                    nc.tensor.matmul(
                                        ps[32 * g:32 * g + 32, :],
                                        wblk[:, wc:wc + 32],
                                        gb[:, g * 2 + h, fs],
                                        start=False, stop=False,
                                        skip_group_check=True,
                                        tile_position=(0, 32 * g),
                                    )
                            u = sb.tile([128, 512], dt.float32, tag="cu",
                                        name="cu")
                            nc.scalar.activation(u[:], ps[:], AF.Copy,
                                                 scale=uscale)
                            nc.vector.scalar_tensor_tensor(
                                out=rout[:, t, fs], in0=rin[:, t, fs],
                                scalar=dcol[:, t:t + 1], in1=u[:],
                                op0=OP.mult, op1=OP.add)
                        nc.sync.dma_start(sl_t[j][:, t, :], rout[:, t, :])
                        if j < K - 1:
                            f8t = sb.tile([128, ROWF], dt.float8e3,
                                          tag="f8t", name="f8t")
                            nc.scalar.activation(f8t[:], rout[:, t, :],
                                                 AF.Copy, scale=0.5)
                            nc.sync.dma_start(sl8_t[j][:, t, :], f8t[:])

            def conv_phase(ci_conv, zring, ln_i, triv, cwsb, sb, tpool, psp):
                """conv + bias + gelu + LN -> zring [128, NT, ROWF] bf16."""
                cb_t = sb.tile([128, C], dt.float32, tag="cvb", name="cvb")
                nc.sync.dma_start(cb_t[:], convb_d.ap()[ci_conv])
                sc, bi = load_ln_consts(ln_i, triv, sb)
                for b in range(B):
                    tT = tpool.tile([128, 2 * K, NOWN], dt.bfloat16, tag="tT",
                                    name="tT")
                    for ci in range(2 * K):
                        j, cb = ci // 2, ci % 2
                        src = sl[j][:, b * 256 + cb * 128:
                                    b * 256 + (cb + 1) * 128]
                        nc.sync.dma_start(tT[:, ci, :], src, transpose=True)
                    for t in range(NT):
                        ps = psp.tile([128, C], dt.float32, tag="cvps",
                                      name="cvps")
                        for ci in range(2 * K):
                            nc.tensor.matmul(
                                ps[:], tT[:, ci, t * 128:(t + 1) * 128],
                                cwsb[:, ci, :],
                                start=(ci == 0), stop=(ci == 2 * K - 1))
                        yt = sb.tile([128, C], dt.float32, tag="cvy",
                                     name="cvy")
                        nc.vector.tensor_tensor(yt[:], ps[:], cb_t[:], OP.add)
                        gf = sb.tile([128, C], dt.float32, tag="cvg",
                                     name="cvg")
                        gelu_act(gf[:], yt[:], sb)
                        ln_norm(gf[:], zring[:, t, b * 256:(b + 1) * 256],
                                ln_i, triv, sb, sc, bi)

            # =========== Phase A: LN1 + chain 1 ===========
            with tc.tile_pool(name="ringA", bufs=1) as ringp, \
                 tc.tile_pool(name="sbA", bufs=3) as sb, \
                 tc.tile_pool(name="gbA", bufs=4) as gbp, \
                 tc.tile_pool(name="psA", bufs=4, space="PSUM") as psp:
                ring = [ringp.tile([128, NT, ROWF], dt.bfloat16,
                                   tag=f"ring{i}", name=f"ring{i}")
                        for i in range(3)]
                sc1, bi1 = load_ln_consts(0, ln_trivial["ln1"], ringp)
                for t in range(NT):
                    xt = sb.tile([128, ROWF], dt.float32, tag="l1x", name="l1x")
                    nc.sync.dma_start(xt[:], maps_t[:, t, :])
                    for b in range(B):
                        ln_norm(xt[:, b * 256:(b + 1) * 256],
                                ring[0][:, t, b * 256:(b + 1) * 256],
                                0, ln_trivial["ln1"], sb, sc1, bi1)
                    nc.sync.dma_start(sl_t[0][:, t, :], ring[0][:, t, :])
                    f8t = sb.tile([128, ROWF], dt.float8e3, tag="f8t",
                                  name="f8t")
                    nc.scalar.activation(f8t[:], ring[0][:, t, :], AF.Copy,
                                         scale=0.5)
                    nc.sync.dma_start(sl8_t[0][:, t, :], f8t[:])
                run_chain(ring, 0, sb, gbp, psp)

            # =========== Phase B: conv1+LN2 -> z1; tokmix -> x1d ===========
            with tc.tile_pool(name="zB", bufs=1) as zp, \
                 tc.tile_pool(name="sbB", bufs=3) as sb:
                z1 = zp.tile([128, NT, ROWF], dt.bfloat16, tag="z1", name="z1")
                with tc.tile_pool(name="cwB", bufs=1) as cwp, \
                     tc.tile_pool(name="tTB", bufs=2) as tpool, \
                     tc.tile_pool(name="psB", bufs=4, space="PSUM") as psp:
                    cwsb = cwp.tile([128, 2 * K, C], dt.bfloat16, tag="cw1",
                                    name="cw1")
                    nc.sync.dma_start(cwsb[:],
                                      cw_d.ap().bitcast(dt.bfloat16)[0])
                    conv_phase(0, z1, 1, ln_trivial["ln2"], cwsb, sb, tpool,
                               psp)

                # token mixing: h1T partials, 8 psum banks
                tmB_ctx = tc.tile_pool(name="tmB", bufs=1)
                tm = tmB_ctx.__enter__()
                w1sb = tm.tile([128, NT, T], dt.bfloat16, tag="w1sb",
                               name="w1sb")
                nc.sync.dma_start(w1sb[:], tokw1_d.ap().bitcast(dt.bfloat16))
                with tc.tile_pool(name="ps8", bufs=1, space="PSUM") as ps8p:
                    ps8 = [ps8p.tile([128, 512], dt.float32, tag=f"h1ps{i}",
                                     name=f"h1ps{i}") for i in range(8)]
                    for t in range(NT):
                        for Tb in range(4):
                            for hf in range(2):
                                nc.tensor.matmul(
                                    ps8[Tb * 2 + hf][:],
                                    w1sb[:, t, Tb * 128:(Tb + 1) * 128],
                                    z1[:, t, hf * 512:(hf + 1) * 512],
                                    start=(t == 0), stop=(t == NT - 1))
                    arsb = tm.tile([128, 4096], dt.float32, tag="arsb",
                                   name="arsb")
                    for i in range(8):
                        nc.scalar.activation(arsb[:, i * 512:(i + 1) * 512],
                                             ps8[i][:], AF.Copy)
                nc.sync.dma_start(arin[:], arsb[:])
                nc.gpsimd.collective_compute(
                    "AllReduce", OP.add, replica_groups=RG,
                    ins=[arin.opt()], outs=[arout.opt()])
                ar2 = tm.tile([128, 4096], dt.float32, tag="ar2", name="ar2")
                nc.sync.dma_start(ar2[:], arout[:])
                b1T = tm.tile([128, 4], dt.float32, tag="b1T", name="b1T")
                nc.sync.dma_start(b1T[:], tokb1T_d.ap())
                h1g = zp.tile([128, 4, ROWF], dt.bfloat16, tag="h1g",
                              name="h1g")
                for Tb in range(4):
                    for hf in range(2):
                        gelu_act(h1g[:, Tb, hf * 512:(hf + 1) * 512],
                                 ar2[:, (Tb * 2 + hf) * 512:
                                     (Tb * 2 + hf + 1) * 512],
                                 sb, bias=b1T[:, Tb:Tb + 1])
                w2sb = zp.tile([128, 4, NOWN], dt.bfloat16, tag="w2sb",
                               name="w2sb")
                nc.sync.dma_start(w2sb[:], tokw2_d.ap().bitcast(dt.bfloat16))
                b2c = zp.tile([128, NT], dt.float32, tag="b2c", name="b2c")
                nc.sync.dma_start(b2c[:], tokb2c_d.ap())
                with tc.tile_pool(name="psB3", bufs=3, space="PSUM") as psp3:
                    for t in range(NT):
                        x1full = sb.tile([128, ROWF], dt.float32, tag="x1f",
                                         name="x1f")
                        for hf in range(2):
                            fs = slice(hf * 512, (hf + 1) * 512)
                            ps = psp3.tile([128, 512], dt.float32, tag="y2ps",
                                           name="y2ps")
                            for Tb in range(4):
                                nc.tensor.matmul(
                                    ps[:], w2sb[:, Tb, t * 128:(t + 1) * 128],
                                    h1g[:, Tb, fs],
                                    start=(Tb == 0), stop=(Tb == 3))
                            mp = sb.tile([128, 512], dt.float32, tag="mp",
                                         name="mp")
                            nc.sync.dma_start(mp[:], maps_t[:, t, fs])
                            nc.vector.scalar_tensor_tensor(
                                out=x1full[:, fs], in0=ps[:],
                                scalar=b2c[:, t:t + 1], in1=mp[:],
                                op0=OP.add, op1=OP.add)
                        nc.sync.dma_start(x1_t[:, t, :], x1full[:])
                tmB_ctx.__exit__(None, None, None)

            # =========== Phase C: LN3 + chain 2 ===========
            with tc.tile_pool(name="ringC", bufs=1) as ringp, \
                 tc.tile_pool(name="sbC", bufs=3) as sb, \
                 tc.tile_pool(name="gbC", bufs=4) as gbp, \
                 tc.tile_pool(name="psC", bufs=4, space="PSUM") as psp:
                ring = [ringp.tile([128, NT, ROWF], dt.bfloat16,
                                   tag=f"rng2{i}", name=f"rng2{i}")
                        for i in range(3)]
                sc3, bi3 = load_ln_consts(2, ln_trivial["ln3"], ringp)
                for t in range(NT):
                    xt = sb.tile([128, ROWF], dt.float32, tag="l3x", name="l3x")
                    nc.sync.dma_start(xt[:], x1_t[:, t, :])
                    for b in range(B):
                        ln_norm(xt[:, b * 256:(b + 1) * 256],
                                ring[0][:, t, b * 256:(b + 1) * 256],
                                2, ln_trivial["ln3"], sb, sc3, bi3)
                    nc.sync.dma_start(sl_t[0][:, t, :], ring[0][:, t, :])
                    f8t = sb.tile([128, ROWF], dt.float8e3, tag="f8t",
                                  name="f8t")
                    nc.scalar.activation(f8t[:], ring[0][:, t, :], AF.Copy,
                                         scale=0.5)
                    nc.sync.dma_start(sl8_t[0][:, t, :], f8t[:])
                run_chain(ring, 5, sb, gbp, psp)

            # =========== Phase D1: conv2 + LN4 -> z4 -> z4d ===========
            with tc.tile_pool(name="zD", bufs=1) as zp, \
                 tc.tile_pool(name="tTD", bufs=2) as tpool, \
                 tc.tile_pool(name="sbD", bufs=3) as sb, \
                 tc.tile_pool(name="psD", bufs=4, space="PSUM") as psp:
                z4 = zp.tile([128, NT, ROWF], dt.bfloat16, tag="z4", name="z4")
                cwsb = zp.tile([128, 2 * K, C], dt.bfloat16, tag="cw2",
                               name="cw2")
                nc.sync.dma_start(cwsb[:], cw_d.ap().bitcast(dt.bfloat16)[1])
                conv_phase(1, z4, 3, ln_trivial["ln4"], cwsb, sb, tpool, psp)
                z4d_t = z4d[:].rearrange("(t p) c -> p t c", p=128)
                for t in range(NT):
                    nc.sync.dma_start(z4d_t[:, t, :], z4[:, t, :])

            # =========== Phase D2: channel mixing + residual ===========
            with tc.tile_pool(name="cmE", bufs=1) as zp, \
                 tc.tile_pool(name="sbE", bufs=3) as sb, \
                 tc.tile_pool(name="hE", bufs=2) as hp, \
                 tc.tile_pool(name="psE", bufs=3, space="PSUM") as psp, \
                 tc.tile_pool(name="psE2", bufs=3, space="PSUM") as psp2:
                w1cs = zp.tile([128, 2, CHID], dt.bfloat16, tag="w1c",
                               name="w1c")
                nc.sync.dma_start(w1cs[:], chw1_d.ap().bitcast(dt.bfloat16))
                w2cs = zp.tile([128, 8, C], dt.bfloat16, tag="w2c", name="w2c")
                nc.sync.dma_start(w2cs[:], chw2_d.ap().bitcast(dt.bfloat16))
                b1c = zp.tile([128, 8], dt.float32, tag="b1c", name="b1c")
                nc.sync.dma_start(b1c[:], chb1_d.ap())
                b2bc = zp.tile([128, C], dt.float32, tag="b2bc", name="b2bc")
                nc.sync.dma_start(b2bc[:], chb2bc_d.ap())
                z4T = zp.tile([128, B, 2, NOWN], dt.bfloat16, tag="z4T",
                              name="z4T")
                for b in range(B):
                    for cb in range(2):
                        src = z4d[:, b * 256 + cb * 128:b * 256 + (cb + 1) * 128]
                        nc.sync.dma_start(z4T[:, b, cb, :], src, transpose=True)
                for b in range(B):
                    for nch in range(3):
                        ns = slice(nch * 512, (nch + 1) * 512)
                        h = hp.tile([128, 8, 512], dt.bfloat16, tag="hcm",
                                    name="hcm")
                        for co8 in range(8):
                            psh = psp.tile([128, 512], dt.float32, tag="psh",
                                           name="psh")
                            for cb in range(2):
                                nc.tensor.matmul(
                                    psh[:],
                                    w1cs[:, cb, co8 * 128:(co8 + 1) * 128],
                                    z4T[:, b, cb, ns],
                                    start=(cb == 0), stop=(cb == 1))
                            gelu_act(h[:, co8, :], psh[:], sb,
                                     bias=b1c[:, co8:co8 + 1])
                        for nt4 in range(4):
                            t = nch * 4 + nt4
                            pso = psp2.tile([128, C], dt.float32, tag="pso",
                                            name="pso")
                            for co8 in range(8):
                                nc.tensor.matmul(
                                    pso[:],
                                    h[:, co8, nt4 * 128:(nt4 + 1) * 128],
                                    w2cs[:, co8, :],
                                    start=(co8 == 0), stop=(co8 == 7))
                            x1t = sb.tile([128, C], dt.float32, tag="x1t",
                                          name="x1t")
                            nc.sync.dma_start(
                                x1t[:], x1_t[:, t, b * 256:(b + 1) * 256])
                            ot = sb.tile([128, C], dt.float32, tag="ot",
                                         name="ot")
                            nc.vector.tensor_tensor(ot[:], pso[:], x1t[:],
                                                    OP.add)
                            nc.vector.tensor_tensor(ot[:], ot[:], b2bc[:],
                                                    OP.add)
                            nc.sync.dma_start(
                                outp_t[:, t, b * 256:(b + 1) * 256], ot[:])

    nc.compile()
    return nc


def kernel(**inputs):
    from concourse import bass_utils
    in_maps, ln_trivial = host_prep(inputs)

    key = ("nc8v2", tuple(sorted(ln_trivial.items())))
    if key not in _cache:
        _cache[key] = build_nc(NCORES, ln_trivial)
    nc = _cache[key]

    res = bass_utils.run_bass_kernel_spmd(nc, in_maps,
                                          core_ids=list(range(NCORES)))

    out = np.zeros((B, N, C), np.float32)
    for q in range(NCORES):
        out[:, q * NOWN:(q + 1) * NOWN, :] = (
            res.results[q]["outp"].reshape(NOWN, B, C).transpose(1, 0, 2))
    return out

